# revision 11
# baseline (speedup 1.0000x reference)
"""CTC total-score (log-semiring) kernel for Trainium2, data-parallel over 8 NeuronCores.

Algorithm: linear-domain CTC forward DP with per-state block-floating-point
scales (refreshed every RWIN steps via ACT Ln/Exp), a +LAM emission bias to
cancel drift, and a "freeze-at-the-answer" emission rewrite so every core runs
an identical 799-step program (all per-utterance variation lives in data).

Per core (4 utterances):
  - stream [128, 5000] frame tiles from HBM, gather the 2x208 extended-label
    channels per frame on GPSIMD (ap_gather), exp(x+LAM) on ScalarE, round-trip
    through DRAM into [4utt, 8steps x 416] chunks.
  - DVE runs the 6-op/step recurrence:
      A'[s] = E_t[s]*(A[s] + F1[s]*A[s-1]) + Em_t[s]*F2[s]*A[s-2]
    with F1/F2 = exp(C[s-1]-C[s]), exp(C[s-2]-C[s]) per-state scale factors.
  - final A and C rows are DMA'd out; host does a tiny f64 epilogue.
"""

import os
import sys
import numpy as np

for _p in ("/root/.axon_site", "/root/.axon_site/_ro/trn_rl_repo",
           "/root/.axon_site/_ro/pypackages", "/opt/trn_rl_repo"):
    if os.path.isdir(_p) and _p not in sys.path:
        sys.path.append(_p)

# ---- problem constants (hardcoded; harness contract) ----
B, T, CC, LL = 32, 800, 5000, 100
NCORES = 8
LAM = 5.5
RWIN = 8
FLOOR = 1e-38
DCLAMP = 60.0
NEGPAD = -300.0     # sentinel channel CC   -> E = exp(-300+5.5) = 0
ONEPAD = -5.5       # sentinel channel CC+1 -> E = exp(-5.5+5.5) = 1

_CACHE = {}


def _derived(upc, t_len, cc, ll):
    s = 2 * ll + 1
    spad = ((s + 7) // 8) * 8          # per-half index padding (208 for L=100)
    nidx = 2 * spad                    # gather width (E half + Em half)
    nw = nidx // 16                    # wrapped idx columns per group
    rows = upc * t_len
    nt = rows // 128                   # gather tiles per core
    nch = t_len // RWIN                # DP chunks of RWIN steps
    return s, spad, nidx, nw, rows, nt, nch


def build_bass(upc=4, t_len=T, cc=CC, ll=LL, n_devices=NCORES):
    """Builds the (input-independent) Bass/Tile program."""
    from contextlib import ExitStack
    import concourse.bacc as bacc
    import concourse.mybir as mybir
    import concourse.tile as tile

    s, spad, nidx, nw, rows, nt, nch = _derived(upc, t_len, cc, ll)
    cp = cc + 2
    f32 = mybir.dt.float32
    i16 = mybir.dt.int16
    AF = mybir.ActivationFunctionType
    assert rows % 128 == 0 and t_len % 16 == 0 and nidx % 16 == 0

    nc = bacc.Bacc("TRN2", target_bir_lowering=False, debug=False,
                   num_devices=n_devices)
    x = nc.dram_tensor("x", [rows, cc], f32, kind="ExternalInput").ap()
    idx = nc.dram_tensor("idx", [128, nt * nw], i16, kind="ExternalInput").ap()
    fin = nc.dram_tensor("fin", [8, s + 3], f32, kind="ExternalOutput").ap()

    # gather tile g -> earliest DP chunk that consumes any of its rows
    def tile_priority(g):
        pr = nch
        for j in range(8):
            base = 128 * g + 16 * j
            tl0 = base % t_len
            pr = min(pr, tl0 // RWIN)
        return pr
    order = sorted(range(nt), key=tile_priority)

    with tile.TileContext(nc) as tc:
        with ExitStack() as ctx:
            xpool = ctx.enter_context(tc.tile_pool(name="xt", bufs=3))
            gpool = ctx.enter_context(tc.tile_pool(name="gt", bufs=2))
            epool = ctx.enter_context(tc.tile_pool(name="et", bufs=2))
            egd = ctx.enter_context(tc.tile_pool(name="egd", bufs=nt, space="DRAM"))
            chpool = ctx.enter_context(tc.tile_pool(name="ch", bufs=3))
            small = ctx.enter_context(tc.tile_pool(name="sm", bufs=1))
            idxp = ctx.enter_context(tc.tile_pool(name="ix", bufs=1))

            idx_sb = idxp.tile([128, nt * nw], i16)
            nc.sync.dma_start(idx_sb[:], idx[:])

            # persistent DP state
            abuf = small.tile([upc, s + 3], f32, tag="abuf")
            cbuf = small.tile([upc, s + 3], f32, tag="cbuf")
            f1 = small.tile([upc, s], f32, tag="f1")
            f2 = small.tile([upc, s], f32, tag="f2")
            xw = small.tile([upc, s], f32, tag="xw")
            zw = small.tile([upc, s], f32, tag="zw")
            t1b = small.tile([upc, s], f32, tag="t1b")
            lab = small.tile([upc, s], f32, tag="lab")
            d1 = small.tile([upc, s], f32, tag="d1")
            d2 = small.tile([upc, s], f32, tag="d2")
            ones = small.tile([upc, 16], f32, tag="ones")
            bias_lam = small.tile([128, 1], f32, tag="bias_lam")

            nc.gpsimd.memset(bias_lam[:], LAM)
            nc.gpsimd.memset(abuf[:], 0.0)
            nc.gpsimd.memset(cbuf[:], 0.0)
            nc.gpsimd.memset(f1[:], 1.0)
            nc.gpsimd.memset(f2[:], 1.0)
            nc.gpsimd.memset(ones[:], 1.0)

            # ---- phase A: stream + gather + exp + spill ----
            eg_tiles = [None] * nt
            for g in range(nt):
                eg_tiles[g] = egd.tile([128, nidx], f32, tag="eg",
                                       name=f"eg{g}")
            for g in order:
                xt = xpool.tile([128, cp], f32, tag="xt")
                nc.sync.dma_start(xt[:, 0:cc], x[128 * g:128 * (g + 1), :])
                nc.gpsimd.memset(xt[:, cc:cc + 1], NEGPAD)
                nc.gpsimd.memset(xt[:, cc + 1:cc + 2], ONEPAD)
                gt = gpool.tile([128, nidx], f32, tag="gt")
                nc.gpsimd.ap_gather(gt[:], xt[:, 0:cp],
                                    idx_sb[:, g * nw:(g + 1) * nw],
                                    channels=128, num_elems=cp, d=1,
                                    num_idxs=nidx)
                et = epool.tile([128, nidx], f32, tag="et")
                nc.scalar.activation(et[:], gt[:], AF.Exp, bias=bias_lam[:])
                nc.sync.dma_start(eg_tiles[g][:], et[:])

            # ---- phase B: DP ----
            for c in range(nch):
                ech = chpool.tile([upc, RWIN * nidx], f32, tag="ech")
                for u in range(upc):
                    r0 = u * t_len + RWIN * c
                    g = r0 // 128
                    o = r0 % 128
                    nc.sync.dma_start(ech[u:u + 1, :],
                                      eg_tiles[g][o:o + RWIN, :])
                if c == 0:
                    # init: A[0,1] = E_0[0,1]
                    nc.vector.tensor_copy(abuf[:, 2:4], ech[:, 0:2])
                for tl in range(RWIN):
                    t = RWIN * c + tl
                    if t == 0:
                        continue
                    co = tl * nidx
                    e_t = ech[:, co:co + s]
                    em_t = ech[:, co + spad:co + spad + s]
                    # A'[s] = E*(A + F1*shift1(A)) + Em*F2*shift2(A)
                    nc.vector.tensor_mul(xw[:], f1[:], abuf[:, 1:1 + s])
                    nc.vector.tensor_add(xw[:], xw[:], abuf[:, 2:2 + s])
                    nc.vector.tensor_mul(xw[:], xw[:], e_t)
                    nc.vector.tensor_mul(zw[:], f2[:], abuf[:, 0:s])
                    nc.vector.tensor_mul(zw[:], zw[:], em_t)
                    nc.vector.tensor_add(abuf[:, 2:2 + s], xw[:], zw[:])
                    if t % RWIN == RWIN - 1 and t != t_len - 1:
                        # refresh per-state scales
                        nc.vector.tensor_scalar_max(t1b[:], abuf[:, 2:2 + s], FLOOR)
                        nc.scalar.activation(lab[:], t1b[:], AF.Ln)
                        nc.vector.tensor_add(cbuf[:, 2:2 + s], cbuf[:, 2:2 + s], lab[:])
                        e = min(2 * t + 1, s - 1)
                        if e < s - 1:
                            w = min(e + 17, s) - (e + 1)
                            nc.vector.tensor_scalar_mul(
                                cbuf[:, 3 + e:3 + e + w], ones[:, 0:w],
                                cbuf[:, 2 + e:3 + e])
                        nc.vector.tensor_copy(cbuf[:, 0:2], cbuf[:, 2:4])
                        nc.vector.tensor_sub(d1[:], cbuf[:, 1:1 + s], cbuf[:, 2:2 + s])
                        nc.vector.tensor_scalar_min(d1[:], d1[:], DCLAMP)
                        nc.vector.tensor_sub(d2[:], cbuf[:, 0:s], cbuf[:, 2:2 + s])
                        nc.vector.tensor_scalar_min(d2[:], d2[:], DCLAMP)
                        nc.scalar.activation(f1[:], d1[:], AF.Exp)
                        nc.scalar.activation(f2[:], d2[:], AF.Exp)
                        ntouch = min(2 * t + 2, s)
                        nc.gpsimd.memset(abuf[:, 2:2 + ntouch], 1.0)

            nc.sync.dma_start(fin[0:upc, :], abuf[:])
            nc.sync.dma_start(fin[4:4 + upc, :], cbuf[:])
    nc.finalize()
    return nc


# ---------------- host-side data prep ----------------

def build_idx_host(targets_u, ilen_u, tlen_u, t_len=T, cc=CC, ll=LL):
    """Per-utterance E/Em gather index vectors + the post-freeze variant."""
    s, spad, nidx, nw, _, _, _ = _derived(1, t_len, cc, ll)
    ext = np.zeros(spad, np.int64)
    ext[1:s:2] = targets_u
    ext_pad = ext.copy()
    ext_pad[s:] = cc                      # sentinel -> E=0
    extm2 = np.full(spad, -1, np.int64)
    extm2[2:] = ext_pad[:-2]
    skip = (ext_pad != 0) & (ext_pad != extm2)
    idx2 = np.where(skip, ext_pad, cc)
    idx2[s:] = cc
    sl = 2 * int(tlen_u)
    post_e = np.full(spad, cc, np.int64)
    post_e[sl] = cc + 1                   # E=1 only at the absorbing state
    post_m = np.full(spad, cc, np.int64)
    v_norm = np.concatenate([ext_pad, idx2]).astype(np.int16)
    v_post = np.concatenate([post_e, post_m]).astype(np.int16)
    return v_norm, v_post


def make_core_inputs(nnet, targets, ilens, tlens, k, upc=4, t_len=T, cc=CC, ll=LL):
    """Builds (x, idx) arrays for core k. x is a copy with freeze-row edits."""
    s, spad, nidx, nw, rows, nt, nch = _derived(upc, t_len, cc, ll)
    u0 = k * upc
    xc = np.ascontiguousarray(
        nnet[u0:u0 + upc].reshape(rows, cc)).copy()
    idx_host = np.zeros((128, nt * nw), np.int16)
    vecs = []
    for j in range(upc):
        u = u0 + j
        v_norm, v_post = build_idx_host(targets[u], ilens[u], tlens[u],
                                        t_len, cc, ll)
        vecs.append((v_norm, v_post))
        # freeze edits: rows [len_u, ceil16(len_u)) -> all NEGPAD except ch0=ONEPAD
        tu = int(ilens[u])
        g_end = ((tu + 15) // 16) * 16
        for fr in range(tu, min(g_end, t_len)):
            xc[j * t_len + fr, :] = NEGPAD
            xc[j * t_len + fr, 0] = ONEPAD
    for g in range(nt):
        for j8 in range(8):
            base = 128 * g + 16 * j8
            u = base // t_len
            tl0 = base % t_len
            v_norm, v_post = vecs[u]
            v = v_post if tl0 >= int(ilens[u0 + u]) else v_norm
            wrapped = v.reshape(nw, 16).T          # [16, nw]: idx i at [i%16, i//16]
            idx_host[16 * j8:16 * (j8 + 1), g * nw:(g + 1) * nw] = wrapped
    return xc, idx_host


def epilogue(fins, ilens, tlens, upc=4, t_len=T, ll=LL, lam=LAM):
    """fins: list of per-core [8, s+3] arrays -> (tot_score, tot_frames, all_frames)."""
    s = 2 * ll + 1
    tots = np.zeros(len(fins) * upc, np.float64)
    for k, f in enumerate(fins):
        for j in range(upc):
            u = k * upc + j
            sl = 2 * int(tlens[u])
            a_prev, a_last = np.float64(f[j, 1 + sl]), np.float64(f[j, 2 + sl])
            c_prev, c_last = np.float64(f[4 + j, 1 + sl]), np.float64(f[4 + j, 2 + sl])
            xp = c_prev + np.log(a_prev) if a_prev > 0 else -np.inf
            xl = c_last + np.log(a_last) if a_last > 0 else -np.inf
            m = max(xp, xl)
            if np.isfinite(m):
                tots[u] = m + np.log(np.exp(xp - m) + np.exp(xl - m)) \
                    - np.float64(lam) * int(ilens[u])
            else:
                tots[u] = -np.inf
    finite = tots > (-1e30 / 2)
    tots32 = tots.astype(np.float32)
    tot_score = np.float32(np.sum(np.where(finite, tots32, np.float32(0.0)),
                                  dtype=np.float32))
    frames = ilens.astype(np.int64)
    tot_frames = np.int32(np.sum(np.where(finite, frames, 0)))
    all_frames = np.int32(np.sum(frames))
    return tot_score, tot_frames, all_frames


def kernel(nnet_output, targets, input_lengths, target_lengths):
    nnet = np.asarray(nnet_output, np.float32)
    targets = np.asarray(targets, np.int32)
    ilens = np.asarray(input_lengths, np.int32)
    tlens = np.asarray(target_lengths, np.int32)
    assert nnet.shape == (B, T, CC) and targets.shape == (B, LL)

    from concourse import bass_utils

    if "nc" not in _CACHE:
        _CACHE["nc"] = build_bass()
    nc = _CACHE["nc"]

    in_maps = []
    for k in range(NCORES):
        xc, idx_host = make_core_inputs(nnet, targets, ilens, tlens, k)
        in_maps.append({"x": xc, "idx": idx_host})

    trace = bool(int(os.environ.get("KERNEL_TRACE", "0")))
    res = bass_utils.run_bass_kernel_spmd(nc, in_maps,
                                          core_ids=list(range(NCORES)),
                                          trace=trace)
    _CACHE["last_result"] = res
    fins = [r["fin"] for r in res.results]
    tot_score, tot_frames, all_frames = epilogue(fins, ilens, tlens)
    return (np.float32(tot_score), np.int32(tot_frames), np.int32(all_frames))


# revision 16
# speedup vs baseline: 1.1335x; 1.1335x over previous
"""CTC total-score (log-semiring) kernel for Trainium2, data-parallel over 8 NeuronCores.

Algorithm: linear-domain CTC forward DP with per-state block-floating-point
scales (refreshed every RWIN steps via ACT Ln/Exp), a +LAM emission bias to
cancel drift, and a "freeze-at-the-answer" emission rewrite so every core runs
an identical 799-step program (all per-utterance variation lives in data).

Per core (4 utterances):
  - stream [128, 5000] frame tiles from HBM, gather the 2x208 extended-label
    channels per frame on GPSIMD (ap_gather), exp(x+LAM) on ScalarE, round-trip
    through DRAM into [4utt, 8steps x 416] chunks.
  - DVE runs the 6-op/step recurrence:
      A'[s] = E_t[s]*(A[s] + F1[s]*A[s-1]) + Em_t[s]*F2[s]*A[s-2]
    with F1/F2 = exp(C[s-1]-C[s]), exp(C[s-2]-C[s]) per-state scale factors.
  - final A and C rows are DMA'd out; host does a tiny f64 epilogue.
"""

import os
import sys
import numpy as np

for _p in ("/root/.axon_site", "/root/.axon_site/_ro/trn_rl_repo",
           "/root/.axon_site/_ro/pypackages", "/opt/trn_rl_repo"):
    if os.path.isdir(_p) and _p not in sys.path:
        sys.path.append(_p)

# ---- problem constants (hardcoded; harness contract) ----
B, T, CC, LL = 32, 800, 5000, 100
NCORES = 8
LAM = 5.5
RWIN = 8
FLOOR = 1e-38
DCLAMP = 60.0
NEGPAD = -300.0     # sentinel channel CC   -> E = exp(-300+5.5) = 0
ONEPAD = -5.5       # sentinel channel CC+1 -> E = exp(-5.5+5.5) = 1

_CACHE = {}


def _derived(upc, t_len, cc, ll):
    s = 2 * ll + 1
    spad = ((s + 7) // 8) * 8          # per-half index padding (208 for L=100)
    nidx = 2 * spad                    # gather width (E half + Em half)
    nw = nidx // 16                    # wrapped idx columns per group
    rows = upc * t_len
    nt = rows // 128                   # gather tiles per core
    nch = t_len // RWIN                # DP chunks of RWIN steps
    return s, spad, nidx, nw, rows, nt, nch


def build_bass(upc=4, t_len=T, cc=CC, ll=LL, n_devices=NCORES):
    """Builds the (input-independent) Bass/Tile program."""
    from contextlib import ExitStack
    import concourse.bacc as bacc
    import concourse.mybir as mybir
    import concourse.tile as tile

    s, spad, nidx, nw, rows, nt, nch = _derived(upc, t_len, cc, ll)
    cp = cc + 2
    f32 = mybir.dt.float32
    i16 = mybir.dt.int16
    AF = mybir.ActivationFunctionType
    assert rows % 128 == 0 and t_len % 16 == 0 and nidx % 16 == 0

    nc = bacc.Bacc("TRN2", target_bir_lowering=False, debug=False,
                   num_devices=n_devices)

    # Pin all ACT activations to the one table set containing Exp+Ln+Copy so
    # the refresh loop doesn't thrash ACT_TABLE_LOAD (1.3us each). Emptying
    # the competing sets preserves list positions (= act_func_set ids).
    from concourse.hw_specs import get_activation_tables as _gat
    _orig_tabs = _gat(nc.m.arch)
    _keep = "natural_log_exp_and_others"
    if _keep in _orig_tabs:
        _filtered = {
            name: (s if name == _keep
                   else (frozenset() if (AF.Exp in s or AF.Ln in s) else s))
            for name, s in _orig_tabs.items()
        }
        bacc.get_activation_tables = lambda arch: _filtered
    x = nc.dram_tensor("x", [rows, cc], f32, kind="ExternalInput").ap()
    idx = nc.dram_tensor("idx", [128, nt * nw], i16, kind="ExternalInput").ap()
    fin = nc.dram_tensor("fin", [8, s + 3], f32, kind="ExternalOutput").ap()

    # gather tile g -> earliest DP chunk that consumes any of its rows
    def tile_priority(g):
        pr = nch
        for j in range(8):
            base = 128 * g + 16 * j
            tl0 = base % t_len
            pr = min(pr, tl0 // RWIN)
        return pr
    order = sorted(range(nt), key=tile_priority)

    with tile.TileContext(nc) as tc:
        with ExitStack() as ctx:
            xpool = ctx.enter_context(tc.tile_pool(name="xt", bufs=3))
            gpool = ctx.enter_context(tc.tile_pool(name="gt", bufs=2))
            epool = ctx.enter_context(tc.tile_pool(name="et", bufs=2))
            egd = ctx.enter_context(tc.tile_pool(name="egd", bufs=nt, space="DRAM"))
            chpool = ctx.enter_context(tc.tile_pool(name="ch", bufs=3))
            small = ctx.enter_context(tc.tile_pool(name="sm", bufs=1))
            idxp = ctx.enter_context(tc.tile_pool(name="ix", bufs=1))

            idx_sb = idxp.tile([128, nt * nw], i16)
            nc.sync.dma_start(idx_sb[:], idx[:])

            # persistent DP state
            abuf = small.tile([upc, s + 3], f32, tag="abuf")
            cbuf = small.tile([upc, s + 3], f32, tag="cbuf")
            f1 = small.tile([upc, s], f32, tag="f1")
            f2 = small.tile([upc, s], f32, tag="f2")
            xw = small.tile([upc, s], f32, tag="xw")
            zw = small.tile([upc, s], f32, tag="zw")
            lab = small.tile([upc, s], f32, tag="lab")
            d1 = small.tile([upc, s], f32, tag="d1")
            d2 = small.tile([upc, s], f32, tag="d2")
            ones = small.tile([upc, 16], f32, tag="ones")
            bias_lam = small.tile([128, 1], f32, tag="bias_lam")
            bias_floor = small.tile([128, 1], f32, tag="bias_floor")

            nc.gpsimd.memset(bias_lam[:], LAM)
            nc.gpsimd.memset(bias_floor[:], FLOOR)
            nc.gpsimd.memset(abuf[:], 0.0)
            nc.gpsimd.memset(cbuf[:], 0.0)
            nc.gpsimd.memset(f1[:], 1.0)
            nc.gpsimd.memset(f2[:], 1.0)
            nc.gpsimd.memset(ones[:], 1.0)

            # ---- phase A: stream + gather + exp + spill ----
            eg_tiles = [None] * nt
            for g in range(nt):
                eg_tiles[g] = egd.tile([128, nidx], f32, tag="eg",
                                       name=f"eg{g}")
            for g in order:
                xt = xpool.tile([128, cp], f32, tag="xt")
                nc.sync.dma_start(xt[:, 0:cc], x[128 * g:128 * (g + 1), :])
                nc.gpsimd.memset(xt[:, cc:cc + 1], NEGPAD)
                nc.gpsimd.memset(xt[:, cc + 1:cc + 2], ONEPAD)
                gt = gpool.tile([128, nidx], f32, tag="gt")
                nc.gpsimd.ap_gather(gt[:], xt[:, 0:cp],
                                    idx_sb[:, g * nw:(g + 1) * nw],
                                    channels=128, num_elems=cp, d=1,
                                    num_idxs=nidx)
                et = epool.tile([128, nidx], f32, tag="et")
                nc.scalar.activation(et[:], gt[:], AF.Exp, bias=bias_lam[:])
                nc.sync.dma_start(eg_tiles[g][:], et[:])

            # ---- phase B: DP ----
            CHK = 16
            for c in range(t_len // CHK):
                ech = chpool.tile([upc, CHK * nidx], f32, tag="ech")
                for u in range(upc):
                    r0 = u * t_len + CHK * c
                    g = r0 // 128
                    o = r0 % 128
                    nc.sync.dma_start(ech[u:u + 1, :],
                                      eg_tiles[g][o:o + CHK, :])
                if c == 0:
                    # init: A[0,1] = E_0[0,1]
                    nc.vector.tensor_copy(abuf[:, 2:4], ech[:, 0:2])
                for tl in range(CHK):
                    t = CHK * c + tl
                    if t == 0:
                        continue
                    co = tl * nidx
                    e_t = ech[:, co:co + s]
                    em_t = ech[:, co + spad:co + spad + s]
                    # A'[s] = E*(A + F1*shift1(A)) + Em*F2*shift2(A)
                    nc.vector.tensor_mul(xw[:], f1[:], abuf[:, 1:1 + s])
                    nc.vector.tensor_add(xw[:], xw[:], abuf[:, 2:2 + s])
                    nc.vector.tensor_mul(xw[:], xw[:], e_t)
                    nc.vector.tensor_mul(zw[:], f2[:], abuf[:, 0:s])
                    nc.vector.tensor_mul(zw[:], zw[:], em_t)
                    nc.vector.tensor_add(abuf[:, 2:2 + s], xw[:], zw[:])
                    if t % RWIN == RWIN - 1 and t != t_len - 1:
                        # refresh per-state scales; Ln(A + FLOOR) via ACT bias
                        nc.scalar.activation(lab[:], abuf[:, 2:2 + s], AF.Ln,
                                             bias=bias_floor[0:upc])
                        nc.vector.tensor_add(cbuf[:, 2:2 + s], cbuf[:, 2:2 + s], lab[:])
                        e = min(2 * t + 1, s - 1)
                        if e < s - 1:
                            w = min(e + 17, s) - (e + 1)
                            nc.vector.tensor_scalar_mul(
                                cbuf[:, 3 + e:3 + e + w], ones[:, 0:w],
                                cbuf[:, 2 + e:3 + e])
                        nc.vector.tensor_copy(cbuf[:, 0:2], cbuf[:, 2:4])
                        nc.vector.tensor_sub(d1[:], cbuf[:, 1:1 + s], cbuf[:, 2:2 + s])
                        nc.vector.tensor_scalar_min(d1[:], d1[:], DCLAMP)
                        nc.vector.tensor_sub(d2[:], cbuf[:, 0:s], cbuf[:, 2:2 + s])
                        nc.vector.tensor_scalar_min(d2[:], d2[:], DCLAMP)
                        nc.scalar.activation(f1[:], d1[:], AF.Exp)
                        nc.scalar.activation(f2[:], d2[:], AF.Exp)
                        ntouch = min(2 * t + 2, s)
                        nc.gpsimd.memset(abuf[:, 2:2 + ntouch], 1.0)

            nc.sync.dma_start(fin[0:upc, :], abuf[:])
            nc.sync.dma_start(fin[4:4 + upc, :], cbuf[:])
    nc.finalize()
    return nc


# ---------------- host-side data prep ----------------

def build_idx_host(targets_u, ilen_u, tlen_u, t_len=T, cc=CC, ll=LL):
    """Per-utterance E/Em gather index vectors + the post-freeze variant."""
    s, spad, nidx, nw, _, _, _ = _derived(1, t_len, cc, ll)
    ext = np.zeros(spad, np.int64)
    ext[1:s:2] = targets_u
    ext_pad = ext.copy()
    ext_pad[s:] = cc                      # sentinel -> E=0
    extm2 = np.full(spad, -1, np.int64)
    extm2[2:] = ext_pad[:-2]
    skip = (ext_pad != 0) & (ext_pad != extm2)
    idx2 = np.where(skip, ext_pad, cc)
    idx2[s:] = cc
    sl = 2 * int(tlen_u)
    post_e = np.full(spad, cc, np.int64)
    post_e[sl] = cc + 1                   # E=1 only at the absorbing state
    post_m = np.full(spad, cc, np.int64)
    v_norm = np.concatenate([ext_pad, idx2]).astype(np.int16)
    v_post = np.concatenate([post_e, post_m]).astype(np.int16)
    return v_norm, v_post


def make_core_inputs(nnet, targets, ilens, tlens, k, upc=4, t_len=T, cc=CC, ll=LL):
    """Builds (x, idx) arrays for core k. x is a copy with freeze-row edits."""
    s, spad, nidx, nw, rows, nt, nch = _derived(upc, t_len, cc, ll)
    u0 = k * upc
    xc = np.ascontiguousarray(
        nnet[u0:u0 + upc].reshape(rows, cc)).copy()
    idx_host = np.zeros((128, nt * nw), np.int16)
    vecs = []
    for j in range(upc):
        u = u0 + j
        v_norm, v_post = build_idx_host(targets[u], ilens[u], tlens[u],
                                        t_len, cc, ll)
        vecs.append((v_norm, v_post))
        # freeze edits: rows [len_u, ceil16(len_u)) -> all NEGPAD except ch0=ONEPAD
        tu = int(ilens[u])
        g_end = ((tu + 15) // 16) * 16
        for fr in range(tu, min(g_end, t_len)):
            xc[j * t_len + fr, :] = NEGPAD
            xc[j * t_len + fr, 0] = ONEPAD
    for g in range(nt):
        for j8 in range(8):
            base = 128 * g + 16 * j8
            u = base // t_len
            tl0 = base % t_len
            v_norm, v_post = vecs[u]
            v = v_post if tl0 >= int(ilens[u0 + u]) else v_norm
            wrapped = v.reshape(nw, 16).T          # [16, nw]: idx i at [i%16, i//16]
            idx_host[16 * j8:16 * (j8 + 1), g * nw:(g + 1) * nw] = wrapped
    return xc, idx_host


def epilogue(fins, ilens, tlens, upc=4, t_len=T, ll=LL, lam=LAM):
    """fins: list of per-core [8, s+3] arrays -> (tot_score, tot_frames, all_frames)."""
    s = 2 * ll + 1
    tots = np.zeros(len(fins) * upc, np.float64)
    for k, f in enumerate(fins):
        for j in range(upc):
            u = k * upc + j
            sl = 2 * int(tlens[u])
            a_prev, a_last = np.float64(f[j, 1 + sl]), np.float64(f[j, 2 + sl])
            c_prev, c_last = np.float64(f[4 + j, 1 + sl]), np.float64(f[4 + j, 2 + sl])
            xp = c_prev + np.log(a_prev) if a_prev > 0 else -np.inf
            xl = c_last + np.log(a_last) if a_last > 0 else -np.inf
            m = max(xp, xl)
            if np.isfinite(m):
                tots[u] = m + np.log(np.exp(xp - m) + np.exp(xl - m)) \
                    - np.float64(lam) * int(ilens[u])
            else:
                tots[u] = -np.inf
    finite = tots > (-1e30 / 2)
    tots32 = tots.astype(np.float32)
    tot_score = np.float32(np.sum(np.where(finite, tots32, np.float32(0.0)),
                                  dtype=np.float32))
    frames = ilens.astype(np.int64)
    tot_frames = np.int32(np.sum(np.where(finite, frames, 0)))
    all_frames = np.int32(np.sum(frames))
    return tot_score, tot_frames, all_frames


def kernel(nnet_output, targets, input_lengths, target_lengths):
    nnet = np.asarray(nnet_output, np.float32)
    targets = np.asarray(targets, np.int32)
    ilens = np.asarray(input_lengths, np.int32)
    tlens = np.asarray(target_lengths, np.int32)
    assert nnet.shape == (B, T, CC) and targets.shape == (B, LL)

    from concourse import bass_utils

    if "nc" not in _CACHE:
        _CACHE["nc"] = build_bass()
    nc = _CACHE["nc"]

    in_maps = []
    for k in range(NCORES):
        xc, idx_host = make_core_inputs(nnet, targets, ilens, tlens, k)
        in_maps.append({"x": xc, "idx": idx_host})

    trace = bool(int(os.environ.get("KERNEL_TRACE", "0")))
    res = bass_utils.run_bass_kernel_spmd(nc, in_maps,
                                          core_ids=list(range(NCORES)),
                                          trace=trace)
    _CACHE["last_result"] = res
    fins = [r["fin"] for r in res.results]
    tot_score, tot_frames, all_frames = epilogue(fins, ilens, tlens)
    return (np.float32(tot_score), np.int32(tot_frames), np.int32(all_frames))


# revision 20
# speedup vs baseline: 1.5198x; 1.3408x over previous
"""CTC total-score (log-semiring) kernel for Trainium2, data-parallel over 8 NeuronCores.

Algorithm: linear-domain CTC forward DP with per-state block-floating-point
scales (refreshed every RWIN steps via ACT Ln/Exp), a +LAM emission bias to
cancel drift, and a "freeze-at-the-answer" emission rewrite so every core runs
an identical T-1-step program (all per-utterance variation lives in data).

State layout (split for speed): the S=201 states are split into two halves so
the DVE free-dim per op halves:
  half A = states [0, NH-1]          on partitions 0..upc-1
  half B = states [SB0, SPAD-1]      on partitions 32..32+upc-1
Each 6-op step updates both halves in single [36, NH]-span instructions.
B's left edge goes stale by 2 states/step; every refresh its overlap region
[SB0, NH-1] is re-copied from A (C only; the mantissas are reset to 1 anyway).

Per core (4 utterances):
  - stream [128, 5000] frame tiles from HBM, gather the 2x208 extended-label
    channels per frame on GPSIMD (ap_gather), exp(x+LAM) on ScalarE, round-trip
    through DRAM into per-chunk split-layout E tiles.
  - DVE runs the 6-op/step recurrence:
      A'[s] = E_t[s]*(A[s] + F1[s]*A[s-1]) + Em_t[s]*F2[s]*A[s-2]
    with F1/F2 = exp(C[s-1]-C[s]), exp(C[s-2]-C[s]) per-state scale factors.
  - final A and C rows are DMA'd out; host does a tiny f64 epilogue.
"""

import os
import sys
import numpy as np

for _p in ("/root/.axon_site", "/root/.axon_site/_ro/trn_rl_repo",
           "/root/.axon_site/_ro/pypackages", "/opt/trn_rl_repo"):
    if os.path.isdir(_p) and _p not in sys.path:
        sys.path.append(_p)

# ---- problem constants (hardcoded; harness contract) ----
B, T, CC, LL = 32, 800, 5000, 100
NCORES = 8
LAM = 5.5
RWIN = 8
FLOOR = 1e-38
DCLAMP = 60.0
NEGPAD = -300.0     # sentinel channel CC   -> E = exp(-300+5.5) = 0
ONEPAD = -5.5       # sentinel channel CC+1 -> E = exp(-5.5+5.5) = 1
SB0 = 93            # half-B first state (full-size config)
BROW = 32           # half-B partition base (engine ops need 32-aligned bases)

_CACHE = {}


def _derived(upc, t_len, cc, ll):
    s = 2 * ll + 1
    spad = ((s + 7) // 8) * 8          # per-half index padding (208 for L=100)
    nidx = 2 * spad                    # gather width (E half + Em half)
    nw = nidx // 16                    # wrapped idx columns per group
    rows = upc * t_len
    nt = rows // 128                   # gather tiles per core
    return s, spad, nidx, nw, rows, nt


def build_bass(upc=4, t_len=T, cc=CC, ll=LL, n_devices=NCORES,
               rwin=RWIN, sb0=SB0, chk=16):
    """Builds the (input-independent) Bass/Tile program."""
    from contextlib import ExitStack
    import concourse.bacc as bacc
    import concourse.mybir as mybir
    import concourse.tile as tile

    s, spad, nidx, nw, rows, nt = _derived(upc, t_len, cc, ll)
    cp = cc + 2
    nh = spad - sb0                    # width of each half (115 for full size)
    f32 = mybir.dt.float32
    i16 = mybir.dt.int16
    AF = mybir.ActivationFunctionType
    assert rows % 128 == 0 and t_len % 16 == 0 and nidx % 16 == 0
    assert t_len % chk == 0 and chk % rwin == 0
    assert sb0 <= nh                   # halves must jointly cover [0, spad)
    assert sb0 + 2 * rwin + 2 <= nh    # resync erosion stays inside overlap
    span = BROW + upc                  # partition span of split ops (36)

    nc = bacc.Bacc("TRN2", target_bir_lowering=False, debug=False,
                   num_devices=n_devices)

    # Pin all ACT activations to the one table set containing Exp+Ln+Copy so
    # the refresh loop doesn't thrash ACT_TABLE_LOAD (1.3us each). Emptying
    # the competing sets preserves list positions (= act_func_set ids).
    from concourse.hw_specs import get_activation_tables as _gat
    _orig_tabs = _gat(nc.m.arch)
    _keep = "natural_log_exp_and_others"
    if _keep in _orig_tabs:
        _filtered = {
            name: (st if name == _keep
                   else (frozenset() if (AF.Exp in st or AF.Ln in st) else st))
            for name, st in _orig_tabs.items()
        }
        bacc.get_activation_tables = lambda arch: _filtered

    x = nc.dram_tensor("x", [rows, cc], f32, kind="ExternalInput").ap()
    idx = nc.dram_tensor("idx", [128, nt * nw], i16, kind="ExternalInput").ap()
    fin = nc.dram_tensor("fin", [4 * upc, nh + 2], f32,
                         kind="ExternalOutput").ap()

    with tile.TileContext(nc) as tc:
        with ExitStack() as ctx:
            xpool = ctx.enter_context(tc.tile_pool(name="xt", bufs=3))
            gpool = ctx.enter_context(tc.tile_pool(name="gt", bufs=2))
            epool = ctx.enter_context(tc.tile_pool(name="et", bufs=2))
            egd = ctx.enter_context(tc.tile_pool(name="egd", bufs=nt, space="DRAM"))
            chpool = ctx.enter_context(tc.tile_pool(name="ch", bufs=2))
            small = ctx.enter_context(tc.tile_pool(name="sm", bufs=1))
            idxp = ctx.enter_context(tc.tile_pool(name="ix", bufs=1))

            idx_sb = idxp.tile([128, nt * nw], i16)
            nc.sync.dma_start(idx_sb[:], idx[:])

            # persistent DP state, split layout: [span, *]
            abuf = small.tile([span, nh + 2], f32, tag="abuf")
            cbuf = small.tile([span, nh + 2], f32, tag="cbuf")
            f1 = small.tile([span, nh], f32, tag="f1")
            f2 = small.tile([span, nh], f32, tag="f2")
            xw = small.tile([span, nh], f32, tag="xw")
            zw = small.tile([span, nh], f32, tag="zw")
            lab = small.tile([span, nh], f32, tag="lab")
            d1 = small.tile([span, nh], f32, tag="d1")
            d2 = small.tile([span, nh], f32, tag="d2")
            ones = small.tile([span, 16], f32, tag="ones")
            bias_lam = small.tile([128, 1], f32, tag="bias_lam")
            bias_floor = small.tile([128, 1], f32, tag="bias_floor")

            nc.gpsimd.memset(bias_lam[:], LAM)
            nc.gpsimd.memset(bias_floor[:], FLOOR)
            nc.gpsimd.memset(abuf[:], 0.0)
            nc.gpsimd.memset(cbuf[:], 0.0)
            nc.gpsimd.memset(f1[:], 1.0)
            nc.gpsimd.memset(f2[:], 1.0)
            nc.gpsimd.memset(ones[:], 1.0)
            nc.gpsimd.memset(xw[:], 0.0)
            nc.gpsimd.memset(zw[:], 0.0)
            nc.gpsimd.memset(lab[:], 0.0)
            nc.gpsimd.memset(d1[:], 0.0)
            nc.gpsimd.memset(d2[:], 0.0)

            eg_tiles = [None] * nt
            for g in range(nt):
                eg_tiles[g] = egd.tile([128, nidx], f32, tag="eg",
                                       name=f"eg{g}")

            def emit_gather_tile(g):
                xt = xpool.tile([128, cp], f32, tag="xt", name=f"xt{g}")
                nc.sync.dma_start(xt[:, 0:cc], x[128 * g:128 * (g + 1), :])
                nc.gpsimd.memset(xt[:, cc:cc + 1], NEGPAD)
                nc.gpsimd.memset(xt[:, cc + 1:cc + 2], ONEPAD)
                gt = gpool.tile([128, nidx], f32, tag="gt", name=f"gt{g}")
                nc.gpsimd.ap_gather(gt[:], xt[:, 0:cp],
                                    idx_sb[:, g * nw:(g + 1) * nw],
                                    channels=128, num_elems=cp, d=1,
                                    num_idxs=nidx)
                et = epool.tile([128, nidx], f32, tag="et", name=f"et{g}")
                nc.scalar.activation(et[:], gt[:], AF.Exp, bias=bias_lam[:])
                nc.sync.dma_start(eg_tiles[g][:], et[:])

            # ---- interleaved stream + DP ----
            emitted = set()
            for c in range(t_len // chk):
                # make sure the gather tiles this chunk reads are in flight
                for u in range(upc):
                    g = (u * t_len + chk * c) // 128
                    if g not in emitted:
                        emit_gather_tile(g)
                        emitted.add(g)
                ech = chpool.tile([span, chk * 2 * nh], f32, tag="ech")
                # junk rows [upc, BROW) are never DMA'd but are read by the
                # spanning ops; zero the tile so they stay defined/NaN-free
                nc.gpsimd.memset(ech[:], 0.0)
                for u in range(upc):
                    r0 = u * t_len + chk * c
                    g = r0 // 128
                    o = r0 % 128
                    src = eg_tiles[g][o:o + chk, :].rearrange(
                        "t (h c) -> t h c", h=2)
                    # half A -> partition u: E cols [0,nh), Em cols [0,nh)
                    nc.sync.dma_start(ech[u:u + 1, :], src[:, :, 0:nh])
                    # half B -> partition BROW+u: cols [sb0, spad)
                    nc.sync.dma_start(ech[BROW + u:BROW + u + 1, :],
                                      src[:, :, sb0:spad])
                if c == 0:
                    # init: A[0,1] = E_0[0,1] (half A only; B starts at zero)
                    nc.vector.tensor_copy(abuf[0:upc, 2:4], ech[0:upc, 0:2])
                for tl in range(chk):
                    t = chk * c + tl
                    if t == 0:
                        continue
                    co = tl * 2 * nh
                    e_t = ech[:, co:co + nh]
                    em_t = ech[:, co + nh:co + 2 * nh]
                    # A'[s] = E*(A + F1*shift1(A)) + Em*F2*shift2(A)
                    nc.vector.tensor_mul(xw[:], f1[:], abuf[:, 1:1 + nh])
                    nc.vector.tensor_add(xw[:], xw[:], abuf[:, 2:2 + nh])
                    nc.vector.tensor_mul(xw[:], xw[:], e_t)
                    nc.vector.tensor_mul(zw[:], f2[:], abuf[:, 0:nh])
                    nc.vector.tensor_mul(zw[:], zw[:], em_t)
                    nc.vector.tensor_add(abuf[:, 2:2 + nh], xw[:], zw[:])
                    if t % rwin == rwin - 1 and t != t_len - 1:
                        # ---- refresh per-state scales ----
                        nc.scalar.activation(lab[:], abuf[:, 2:2 + nh], AF.Ln,
                                             bias=bias_floor[0:span])
                        nc.vector.tensor_add(cbuf[:, 2:2 + nh],
                                             cbuf[:, 2:2 + nh], lab[:])
                        # resync half-B overlap [sb0, nh-1] from half A
                        nc.vector.tensor_copy(
                            cbuf[BROW:BROW + upc, 2:2 + (nh - sb0)],
                            cbuf[0:upc, 2 + sb0:2 + nh])
                        # wavefront fill of the next arrival zone
                        e = min(2 * t + 1, s - 1)
                        zlo, zhi = e + 1, min(e + 17, s)  # fill states [zlo,zhi)
                        if zlo < zhi:
                            alo, ahi = zlo, min(zhi, nh)
                            if alo < ahi:  # A-part
                                nc.vector.tensor_scalar_mul(
                                    cbuf[0:upc, 2 + alo:2 + ahi],
                                    ones[0:upc, 0:ahi - alo],
                                    cbuf[0:upc, 1 + zlo:2 + zlo])
                            blo, bhi = max(zlo, sb0), zhi
                            if blo < bhi:  # B-part
                                if e >= sb0:
                                    src_ap = cbuf[BROW:BROW + upc,
                                                  2 + e - sb0:3 + e - sb0]
                                else:
                                    # edge state lives only in A: stage a copy
                                    nc.vector.tensor_copy(
                                        cbuf[BROW:BROW + upc, 1 + nh:2 + nh],
                                        cbuf[0:upc, 1 + zlo:2 + zlo])
                                    src_ap = cbuf[BROW:BROW + upc,
                                                  1 + nh:2 + nh]
                                nc.vector.tensor_scalar_mul(
                                    cbuf[BROW:BROW + upc,
                                         2 + blo - sb0:2 + bhi - sb0],
                                    ones[BROW:BROW + upc, 0:bhi - blo],
                                    src_ap)
                        # guard columns follow the half's first two states
                        nc.vector.tensor_copy(cbuf[:, 0:2], cbuf[:, 2:4])
                        # factors
                        nc.vector.tensor_sub(d1[:], cbuf[:, 1:1 + nh],
                                             cbuf[:, 2:2 + nh])
                        nc.vector.tensor_scalar_min(d1[:], d1[:], DCLAMP)
                        nc.vector.tensor_sub(d2[:], cbuf[:, 0:nh],
                                             cbuf[:, 2:2 + nh])
                        nc.vector.tensor_scalar_min(d2[:], d2[:], DCLAMP)
                        nc.scalar.activation(f1[:], d1[:], AF.Exp)
                        nc.scalar.activation(f2[:], d2[:], AF.Exp)
                        # mantissa reset over the touched wavefront
                        ta = min(2 * t + 2, nh)
                        nc.gpsimd.memset(abuf[0:upc, 2:2 + ta], 1.0)
                        tb = min(2 * t + 2, spad) - sb0
                        if tb > 0:
                            nc.gpsimd.memset(
                                abuf[BROW:BROW + upc, 2:2 + tb], 1.0)

            nc.sync.dma_start(fin[0:upc, :], abuf[0:upc, :])
            nc.sync.dma_start(fin[upc:2 * upc, :], abuf[BROW:BROW + upc, :])
            nc.sync.dma_start(fin[2 * upc:3 * upc, :], cbuf[0:upc, :])
            nc.sync.dma_start(fin[3 * upc:4 * upc, :], cbuf[BROW:BROW + upc, :])
    nc.finalize()
    return nc


# ---------------- host-side data prep ----------------

def build_idx_host(targets_u, ilen_u, tlen_u, t_len=T, cc=CC, ll=LL):
    """Per-utterance E/Em gather index vectors + the post-freeze variant."""
    s, spad, nidx, nw, _, _ = _derived(1, t_len, cc, ll)
    ext = np.zeros(spad, np.int64)
    ext[1:s:2] = targets_u
    ext_pad = ext.copy()
    ext_pad[s:] = cc                      # sentinel -> E=0
    extm2 = np.full(spad, -1, np.int64)
    extm2[2:] = ext_pad[:-2]
    skip = (ext_pad != 0) & (ext_pad != extm2)
    idx2 = np.where(skip, ext_pad, cc)
    idx2[s:] = cc
    sl = 2 * int(tlen_u)
    post_e = np.full(spad, cc, np.int64)
    post_e[sl] = cc + 1                   # E=1 only at the absorbing state
    post_m = np.full(spad, cc, np.int64)
    v_norm = np.concatenate([ext_pad, idx2]).astype(np.int16)
    v_post = np.concatenate([post_e, post_m]).astype(np.int16)
    return v_norm, v_post


def make_core_inputs(nnet, targets, ilens, tlens, k, upc=4, t_len=T, cc=CC, ll=LL):
    """Builds (x, idx) arrays for core k. x is a copy with freeze-row edits."""
    s, spad, nidx, nw, rows, nt = _derived(upc, t_len, cc, ll)
    u0 = k * upc
    xc = np.ascontiguousarray(
        nnet[u0:u0 + upc].reshape(rows, cc)).copy()
    idx_host = np.zeros((128, nt * nw), np.int16)
    vecs = []
    for j in range(upc):
        u = u0 + j
        v_norm, v_post = build_idx_host(targets[u], ilens[u], tlens[u],
                                        t_len, cc, ll)
        vecs.append((v_norm, v_post))
        # freeze edits: rows [len_u, ceil16(len_u)) -> all NEGPAD except ch0=ONEPAD
        tu = int(ilens[u])
        g_end = ((tu + 15) // 16) * 16
        for fr in range(tu, min(g_end, t_len)):
            xc[j * t_len + fr, :] = NEGPAD
            xc[j * t_len + fr, 0] = ONEPAD
    for g in range(nt):
        for j8 in range(8):
            base = 128 * g + 16 * j8
            u = base // t_len
            tl0 = base % t_len
            v_norm, v_post = vecs[u]
            v = v_post if tl0 >= int(ilens[u0 + u]) else v_norm
            wrapped = v.reshape(nw, 16).T          # [16, nw]: idx i at [i%16, i//16]
            idx_host[16 * j8:16 * (j8 + 1), g * nw:(g + 1) * nw] = wrapped
    return xc, idx_host


def epilogue(fins, ilens, tlens, upc=4, t_len=T, ll=LL, lam=LAM, sb0=SB0):
    """fins: list of per-core [4*upc, nh+2] arrays (A-A, A-B, C-A, C-B rows)."""
    tots = np.zeros(len(fins) * upc, np.float64)
    for k, f in enumerate(fins):
        for j in range(upc):
            u = k * upc + j
            sl = 2 * int(tlens[u])
            cb = sl - sb0                 # column of state sl in half-B
            a_prev = np.float64(f[upc + j, 1 + cb])
            a_last = np.float64(f[upc + j, 2 + cb])
            c_prev = np.float64(f[3 * upc + j, 1 + cb])
            c_last = np.float64(f[3 * upc + j, 2 + cb])
            xp = c_prev + np.log(a_prev) if a_prev > 0 else -np.inf
            xl = c_last + np.log(a_last) if a_last > 0 else -np.inf
            m = max(xp, xl)
            if np.isfinite(m):
                tots[u] = m + np.log(np.exp(xp - m) + np.exp(xl - m)) \
                    - np.float64(lam) * int(ilens[u])
            else:
                tots[u] = -np.inf
    finite = tots > (-1e30 / 2)
    tots32 = tots.astype(np.float32)
    tot_score = np.float32(np.sum(np.where(finite, tots32, np.float32(0.0)),
                                  dtype=np.float32))
    frames = ilens.astype(np.int64)
    tot_frames = np.int32(np.sum(np.where(finite, frames, 0)))
    all_frames = np.int32(np.sum(frames))
    return tot_score, tot_frames, all_frames


def kernel(nnet_output, targets, input_lengths, target_lengths):
    nnet = np.asarray(nnet_output, np.float32)
    targets = np.asarray(targets, np.int32)
    ilens = np.asarray(input_lengths, np.int32)
    tlens = np.asarray(target_lengths, np.int32)
    assert nnet.shape == (B, T, CC) and targets.shape == (B, LL)
    # half-B must hold the readout states beyond its erosion zone
    assert int(tlens.min()) * 2 - 1 >= SB0 + 2 * RWIN + 2

    from concourse import bass_utils

    if "nc" not in _CACHE:
        _CACHE["nc"] = build_bass()
    nc = _CACHE["nc"]

    in_maps = []
    for k in range(NCORES):
        xc, idx_host = make_core_inputs(nnet, targets, ilens, tlens, k)
        in_maps.append({"x": xc, "idx": idx_host})

    trace = bool(int(os.environ.get("KERNEL_TRACE", "0")))
    res = bass_utils.run_bass_kernel_spmd(nc, in_maps,
                                          core_ids=list(range(NCORES)),
                                          trace=trace)
    _CACHE["last_result"] = res
    fins = [r["fin"] for r in res.results]
    tot_score, tot_frames, all_frames = epilogue(fins, ilens, tlens)
    return (np.float32(tot_score), np.int32(tot_frames), np.int32(all_frames))
